# revision 10
# baseline (speedup 1.0000x reference)
"""Trainium2 Bass kernel for ExponentialConcordanceLoss.

Reference semantics (N = 8192):
    t = targets[:, 0]; e = targets[:, 1] != 0; s = preds
    mask[j, i] = (t[i] < t[j]) & e[i]
    loss = sum_{j,i} mask * exp(s[j] - s[i]) / max(sum(mask), 1)

v5: O(N) suffix-scan factorization. Sorting by t is host-side layout
prep (as in the v3 baseline); every float op on the data runs on
device. With elements laid out in DESCENDING t order (position d),
the inner sum over j collapses to a prefix sum:

    loss_sum = sum_d w_d * P[d] - n_events,  w_d = e_d * exp(-s_d)
    P[d]     = sum_{m <= d} exp(s_m)   (inclusive; the diagonal term
               w_d*exp(s_d) = e_d is removed exactly on the host)
    count    = sum_d e_d * d

Device pipeline per core (all 8 cores run the same static program;
core c's inputs mask w to its slice d in [1024c, 1024(c+1))):
  GPS : v = e^s via tensor_tensor(pow)  (right after the input DMA -
        Pool's Q7 exp is ready before ACT's SBUF-latency exp would be)
        count = sum (u > -1e29) * d  (iota positions)
  DVE : P = tensor_tensor_scan(v)      -> in-row inclusive prefix
        loss = sum (P + R) * w         -> one fused STT with accum
  PE  : R = tri.T @ rowsum (one [128,128] fp32 matmul; tri built
        on-device by GPSIMD iota+compare while the input DMA flies)
  ACT : w = exp(u)  (u = -s masked to event&slice), off critical path
  out : pre-prepared kv_writeback descriptor fired by trigger_dma,
        skipping the DGE fixed latency on the exit path.

Ties in t (strict '<' in the reference) are corrected exactly on the
host from the few affected elements; count is integer-exact.
"""

import sys

if "/opt/trn_rl_repo" not in sys.path:
    sys.path.insert(0, "/opt/trn_rl_repo")

import numpy as np

N = 8192
NCORES = 8
ROWS, COLS = 128, 64  # position d = p*COLS + f (descending t)
IPC = N // NCORES     # positions per core

_CACHE = {}

E_CONST = float(np.exp(np.float64(1.0)))


def _build(trigger_out=True, final_wait=True):
    import concourse.bass as bass
    import concourse.mybir as mybir
    from concourse import library_config

    f32 = mybir.dt.float32
    i32 = mybir.dt.int32
    Alu = mybir.AluOpType
    Act = mybir.ActivationFunctionType

    nc = bass.Bass()

    tin_d = nc.dram_tensor("tin", [ROWS, 2 * COLS], f32, kind="ExternalInput")
    if trigger_out:
        out_d = nc.dram_tensor("out", [1, 128, 1, 2], f32, kind="ExternalOutput")
    else:
        out_d = nc.dram_tensor("out", [128, 2], f32, kind="ExternalOutput")

    from contextlib import ExitStack

    with ExitStack() as ctx:
        en = ctx.enter_context
        tin_s = en(nc.sbuf_tensor([ROWS, 2 * COLS], f32))
        v64 = en(nc.sbuf_tensor([ROWS, COLS], f32))
        p64 = en(nc.sbuf_tensor([ROWS, COLS], f32))
        w = en(nc.sbuf_tensor([ROWS, COLS], f32))
        junk = en(nc.sbuf_tensor([ROWS, COLS], f32))
        junkg = en(nc.sbuf_tensor([ROWS, COLS], f32))
        posd = en(nc.sbuf_tensor([ROWS, COLS], f32))
        econst = en(nc.sbuf_tensor([ROWS, COLS], f32))
        tri_i = en(nc.sbuf_tensor([ROWS, ROWS], f32))
        tri = en(nc.sbuf_tensor([ROWS, ROWS], f32))
        red = en(nc.sbuf_tensor([ROWS, 2], f32))
        actwarm = en(nc.sbuf_tensor([ROWS, 1], f32))
        ctxidx = en(nc.sbuf_tensor([ROWS, 1], i32))
        rp = en(nc.psum_tensor([ROWS, 1], f32))
        dsem = en(nc.semaphore())    # input DMA landed
        asem = en(nc.semaphore())    # ACT exp(u) done
        vsem = en(nc.semaphore())    # Pool v = e^s done
        vv = en(nc.semaphore())      # DVE scan done
        gpsem = en(nc.semaphore())   # tri matrix ready
        gsync = en(nc.semaphore())   # gpsimd intra-engine ordering
        pesem = en(nc.semaphore())   # matmul done
        losssem = en(nc.semaphore())
        outsem = en(nc.semaphore())
        block = en(nc.Block())

        @block.sync
        def _(sync):
            sync.dma_start(tin_s[:], tin_d[:]).then_inc(dsem, 16)
            if not trigger_out:
                sync.wait_ge(losssem, 2)
                sync.dma_start(out_d[:], red[:, 0:2]).then_inc(outsem, 16)
                sync.wait_ge(outsem, 16)

        @block.scalar
        def _(scalar):
            # dummy exp on a const AP preloads the Exp table (~1.3us)
            # while the input DMA is in flight
            scalar.activation(
                actwarm[:], nc.const_aps.scalar_like(0.0, actwarm[:]), Act.Exp
            )
            scalar.wait_ge(dsem, 16)
            scalar.activation(w[:], tin_s[:, COLS : 2 * COLS], Act.Exp).then_inc(
                asem, 1
            )

        @block.vector
        def _(vector):
            # Everything before the scan fits in DVE's dead window while
            # the input DMA is in flight.
            vector.wait_ge(gsync, 3 if trigger_out else 2)
            # tri[q, p] = 1 iff q < p  (strictly-lower in [K=q, M=p] layout)
            vector.tensor_scalar(
                out=tri[:], in0=tri_i[:], scalar1=0.0, scalar2=None,
                op0=Alu.is_gt, op1=Alu.add,
            ).then_inc(gpsem, 1)
            vector.wait_ge(gsync, 4 if trigger_out else 3)
            vector.wait_ge(dsem, 16)
            # count partial: sum over event&slice positions of d
            vector.scalar_tensor_tensor(
                out=junkg[:], in0=tin_s[:, COLS : 2 * COLS], scalar=-1e29,
                in1=posd[:], op0=Alu.is_gt, op1=Alu.mult,
                accum_out=red[:, 1:2],
            ).then_inc(losssem, 1)
            vector.wait_ge(vsem, 1)
            # P[p, f] = sum_{f' <= f} v[p, f'] (inclusive in-row prefix;
            # col 63 is the full row sum)
            vector.tensor_tensor_scan(
                p64[:], v64[:], v64[:], 0.0, Alu.add, Alu.bypass
            ).then_inc(vv, 1)
            vector.wait_ge(asem, 1)
            # ordering after the scan (RAW on p64) is transitive:
            # pesem <- PE matmul <- vv <- scan
            vector.wait_ge(pesem, 1)
            vector.scalar_tensor_tensor(
                out=junk[:], in0=p64[:], scalar=rp[:, 0:1], in1=w[:],
                op0=Alu.add, op1=Alu.mult, accum_out=red[:, 0:1],
            ).then_inc(losssem, 1)

        @block.gpsimd
        def _(gpsimd):
            # kv_writeback + iota + tensor_tensor(pow) all live in 'proxy'
            gpsimd.load_library(library_config.proxy)
            gpsimd.memset(econst[:], E_CONST).then_inc(gsync, 1)
            # tri_i[q, p] = p - q; DVE compares > 0 into tri
            gpsimd.iota(
                tri_i[:], [[1, ROWS]], base=0, channel_multiplier=-1,
                allow_small_or_imprecise_dtypes=True,
            ).then_inc(gsync, 1)
            if trigger_out:
                gpsimd.memset(ctxidx[:], 0).then_inc(gsync, 1)
            gpsimd.iota(
                posd[:], [[1, COLS]], base=0, channel_multiplier=COLS,
                allow_small_or_imprecise_dtypes=True,
            ).then_inc(gsync, 1)
            if trigger_out:
                gpsimd.wait_ge(gsync, 3)
                gpsimd.kv_writeback(
                    out_d[:],
                    bass.AP(red, 0, [[2, 128], [0, 1], [0, 1], [1, 2]]),
                    ctxidx[:],
                    prepare_only=True,
                    sem=outsem,
                )
            gpsimd.wait_ge(gsync, 1)
            gpsimd.wait_ge(dsem, 16)
            # v = e^s on the Q7 (powf) - beats ACT's SBUF access latency
            gpsimd.tensor_tensor(
                out=v64[:], in0=econst[:], in1=tin_s[:, 0:COLS], op=Alu.pow
            ).then_inc(vsem, 1)
            if trigger_out:
                gpsimd.wait_ge(losssem, 2)
                gpsimd.trigger_dma(count=1)
                if final_wait:
                    gpsimd.wait_ge(outsem, 16)

        @block.tensor
        def _(tensor):
            tensor.wait_ge(gpsem, 1)
            tensor.wait_ge(vv, 1)
            # R[p] = sum_{q < p} rowsum[q]
            tensor.matmul(
                rp[:, 0:1], tri[:], p64[:, COLS - 1 : COLS],
                start=True, stop=True,
            ).then_inc(pesem, 1)

    return nc


def _plan(preds, targets):
    """Host layout prep: stable descending-t sort + per-core slice masks.
    Returns (maps, nevents, loss_corr, cnt_corr) or None if no events."""
    t = np.ascontiguousarray(targets[:, 0], dtype=np.float32)
    e = np.ascontiguousarray(targets[:, 1], dtype=np.float32)
    s = np.ascontiguousarray(preds, dtype=np.float32).reshape(-1)

    order = np.argsort(-t, kind="stable")
    td = t[order]
    sd = s[order]
    ed = e[order] != 0.0
    nevents = int(ed.sum())
    if nevents == 0:
        return None

    # Exact tie corrections (strict t_i < t_j in the reference). The
    # device uses positional prefixes; elements inside a tie run of
    # equal t over-count by the run prefix before them.
    loss_corr = 0.0
    cnt_corr = 0
    eq = td[1:] == td[:-1]
    if eq.any():
        starts = np.flatnonzero(np.concatenate([[True], ~eq]))
        run_id = np.concatenate([[0], np.cumsum(~eq)])
        a = starts[run_id]  # a[d] = first position of d's tie run
        affected = np.flatnonzero((a != np.arange(N)) & ed)
        for d in affected:
            aa = int(a[d])
            loss_corr += float(
                np.exp(-np.float64(sd[d]))
                * np.exp(sd[aa:d].astype(np.float64)).sum()
            )
        cnt_corr = int((affected - a[affected]).sum())

    smat = sd.reshape(ROWS, COLS)
    u_full = np.where(ed, -sd, np.float32(-1e30)).astype(np.float32)
    maps = []
    for c in range(NCORES):
        u_c = np.full(N, np.float32(-1e30), np.float32)
        sl = slice(c * IPC, (c + 1) * IPC)
        u_c[sl] = u_full[sl]
        tin = np.empty((ROWS, 2 * COLS), np.float32)
        tin[:, 0:COLS] = smat
        tin[:, COLS:] = u_c.reshape(ROWS, COLS)
        maps.append({"tin": tin})
    return maps, nevents, loss_corr, cnt_corr


def _combine(results, nevents, loss_corr, cnt_corr):
    loss = 0.0
    cnt = 0.0
    for r in results:
        part = np.asarray(r["out"], dtype=np.float64).reshape(128, 2)
        loss += part[:, 0].sum()
        cnt += part[:, 1].sum()
    # remove the inclusive-prefix diagonal (w_d*v_d = e_d) and tie terms
    loss -= nevents + loss_corr
    cnt -= cnt_corr
    return np.array(
        np.float32(loss) / np.float32(max(cnt, 1.0)), dtype=np.float32
    )


def kernel(preds, targets):
    from concourse.bass_utils import run_bass_kernel_spmd

    plan = _plan(np.asarray(preds), np.asarray(targets))
    if plan is None:
        return np.array(0.0, dtype=np.float32)
    maps, nevents, loss_corr, cnt_corr = plan
    if "nc" not in _CACHE:
        _CACHE["nc"] = _build()
    nc = _CACHE["nc"]
    res = run_bass_kernel_spmd(nc, maps, list(range(NCORES)))
    return _combine(res.results, nevents, loss_corr, cnt_corr)
